# revision 1
# baseline (speedup 1.0000x reference)
"""DMN4 topk-masking kernel for Trainium2 (8 NeuronCores, Bass/Tile).

Problem: few-shot episodic loss (DMN4). For each (episode b, query q):
  - cosine similarity S[m, g] between 100 query descriptors (m) and
    2500 support descriptors (g = class w * 500 + shot k * 100 + pos p),
    contracting over c=640 channels.
  - per-class max S_max[w, m]; global argmax "nearest[m]"; top-2 class
    diff; mutual-nearest mask; predict[w] = sum_m S_max*mask*2;
    loss = NLL(log_softmax(predict), y), meaned over all b*q.

Sharding: data-parallel over (b, q). 8 cores = 4 episodes x 2 query
halves; each core processes 38 queries of one episode (cores 2k+1
overlap one query which the host drops when gathering).

Key algorithmic restructurings vs the reference:
  - Query normalization is folded out of the matmul: row scale rq only
    affects the top-2 diff and predict, so it is applied to those tiny
    [100,1] tensors instead of the [640,100] query block.
  - Support column scale rs IS comparison-relevant, so it's applied by
    a fused tensor_tensor_reduce (psum S' * rs -> SBUF) which also
    emits the per-class max in the same pass.
  - The mutual-nearest scatter/gather chain is reformulated as a
    100x100 "same-slot" comparison matrix: winner[m] = first argmax
    over m' of (nearest[m']==nearest[m]) * diff[m'], mask = winner==m.
"""

import numpy as np

from concourse import bacc, bass, mybir
from concourse.bass_utils import run_bass_kernel_spmd
from concourse.masks import make_identity
from concourse.tile import TileContext

DT = mybir.dt
AF = mybir.ActivationFunctionType
OP = mybir.AluOpType

N_WAY = 5
K_SHOT = 5
TEMPERATURE = 2.0
EPS = 1e-8
B, Q, C, HW = 4, 75, 640, 100
MQ = HW            # query descriptors per query image
MS = K_SHOT * HW   # support descriptors per class
NS = N_WAY * MS    # 2500 support descriptors total
CC = C // 128      # 5 chunks of 128 channels
NQ = 38            # queries per core (2 cores x 38 covers 75 with 1 overlap)
NEG = -3.0e38


def build_kernel(mm_dtype=DT.float32r):
    """One SPMD program; every core runs the same 38-query episode slice."""
    nc = bacc.Bacc("TRN2", target_bir_lowering=False, debug=False, num_devices=8)

    sup_d = nc.declare_dram_parameter("sup", [N_WAY * K_SHOT, C, HW], mm_dtype, False)
    qry_d = nc.declare_dram_parameter("qry", [NQ, C, HW], mm_dtype, False)
    oneh_d = nc.declare_dram_parameter("oneh", [1, NQ * N_WAY], DT.float32, False)
    loss_d = nc.declare_dram_parameter("loss", [1, NQ], DT.float32, True)

    def f32(ap):
        """View an mm_dtype AP as plain fp32 for non-matmul consumers."""
        return ap.bitcast(DT.float32) if mm_dtype != DT.float32 else ap

    with TileContext(nc) as tc:
        with (
            tc.tile_pool(name="const", bufs=1) as const,
            tc.tile_pool(name="sup", bufs=1) as supp,
            tc.tile_pool(name="sq", bufs=2) as sqp,
            tc.tile_pool(name="qin", bufs=3) as qin,
            tc.tile_pool(name="sb", bufs=2) as sbp,
            tc.tile_pool(name="small", bufs=3) as sm,
            tc.tile_pool(name="out", bufs=1) as outp,
            tc.tile_pool(name="ps", bufs=5, space="PSUM") as pps,
            tc.tile_pool(name="pt", bufs=3, space="PSUM") as ppt,
        ):
            # ---- constants ----
            ident = const.tile([MQ, MQ], DT.float32, tag="ident")
            make_identity(nc, ident)
            ones1 = const.tile([1, MQ], DT.float32, tag="ones1")
            nc.vector.memset(ones1, 1.0)
            onesc = const.tile([128, 1], mm_dtype, tag="onesc")
            nc.vector.memset(f32(onesc), 1.0)
            onescf = const.tile([128, 1], DT.float32, tag="onescf")
            nc.vector.memset(onescf, 1.0)
            iota_i = const.tile([MQ, 1], DT.int32, tag="iotai")
            nc.gpsimd.iota(iota_i, pattern=[[0, 1]], base=0, channel_multiplier=1)
            iota_f = const.tile([MQ, 1], DT.float32, tag="iotaf")
            nc.vector.tensor_copy(iota_f, iota_i)
            oneh_s = const.tile([1, NQ * N_WAY], DT.float32, tag="oneh")
            nc.sync.dma_start(out=oneh_s, in_=oneh_d[:])

            # ---- support: load [128, cc, w, 500] ----
            sf = supp.tile([128, CC, N_WAY, MS], mm_dtype, tag="sf")
            sup_r = sup_d[:].rearrange(
                "(w k) (cc cp) p -> cp cc w k p", w=N_WAY, cc=CC
            )
            for cc in range(CC):
                for w in range(N_WAY):
                    nc.sync.dma_start(
                        out=sf[:, cc, w].rearrange("cp (k p) -> cp k p", k=K_SHOT),
                        in_=sup_r[:, cc, w],
                    )

            # support norms: sum_c sf^2 via ACT square + ones-matmul, then
            # rs = 1/(sqrt(n2)+eps) broadcast to [100, 2500].
            rs2s = [
                pps.tile([1, MS], DT.float32, tag="sbank", name=f"rs2_{w}")
                for w in range(N_WAY)
            ]
            for cc in range(CC):
                sq = sqp.tile([128, N_WAY, MS], mm_dtype, tag="sq")
                nc.scalar.activation(sq, f32(sf[:, cc]), AF.Square)
                for w in range(N_WAY):
                    nc.tensor.matmul(
                        rs2s[w],
                        onesc,
                        sq[:, w],
                        start=(cc == 0),
                        stop=(cc == CC - 1),
                    )
            rs_row = const.tile([1, NS], DT.float32, tag="rsrow")
            for w in range(N_WAY):
                nc.scalar.activation(rs_row[:, w * MS:(w + 1) * MS], rs2s[w], AF.Sqrt)
            nc.vector.tensor_scalar_add(rs_row, rs_row, EPS)
            nc.vector.reciprocal(rs_row, rs_row)
            ones128 = const.tile([1, 128], DT.float32, tag="ones128")
            nc.vector.memset(ones128, 1.0)
            rs_b = const.tile([128, NS], DT.float32, tag="rsb")
            for w in range(N_WAY):
                rbp = pps.tile([128, MS], DT.float32, tag="sbank")
                nc.tensor.matmul(
                    rbp, ones128, rs_row[:, w * MS:(w + 1) * MS],
                    start=True, stop=True,
                )
                nc.scalar.copy(rs_b[:, w * MS:(w + 1) * MS], rbp)
            # scale support columns in place: sn = sf * rs (f32r out)
            for cc in range(CC):
                nc.vector.tensor_mul(
                    sf[:, cc].rearrange("cp w s -> cp (w s)"),
                    f32(sf[:, cc]).rearrange("cp w s -> cp (w s)"),
                    rs_b,
                )

            prow = outp.tile([1, NQ, N_WAY], DT.float32, tag="prow")

            # ---- per-query main loop ----
            for q in range(NQ):
                qf = qin.tile([128, CC, MQ], mm_dtype, tag="qf")
                nc.sync.dma_start(
                    out=qf, in_=qry_d[q].rearrange("(cc cp) m -> cp cc m", cc=CC)
                )

                # query norm^2 -> [1, 100] psum
                sqq = sqp.tile([128, CC, MQ], mm_dtype, tag="sqq")
                nc.scalar.activation(sqq, f32(qf), AF.Square)
                n2q = ppt.tile([1, MQ], DT.float32, tag="tiny")
                for cc in range(CC):
                    nc.tensor.matmul(
                        n2q, onesc, sqq[:, cc],
                        start=(cc == 0), stop=(cc == CC - 1),
                    )
                # rq2 = 2/(norm+eps): halve+eps as a row, transpose to a
                # column via PE, then reciprocal on [100,1] (cheap on DVE)
                rq2r = sm.tile([1, MQ], DT.float32, tag="rq2r")
                nc.scalar.activation(rq2r, n2q, AF.Sqrt)
                nc.vector.tensor_scalar(
                    rq2r, rq2r, 0.5, EPS * 0.5, op0=OP.mult, op1=OP.add
                )
                rq2p = ppt.tile([MQ, 1], DT.float32, tag="tiny")
                nc.tensor.matmul(rq2p, rq2r, onescf[0:1, :], start=True, stop=True)
                rq2 = sm.tile([MQ, 1], DT.float32, tag="rq2")
                nc.scalar.copy(rq2, rq2p)
                nc.vector.reciprocal(rq2, rq2)

                # S' = qf^T sf (per class bank), fused scale+max via TTR
                s_sb = sbp.tile([MQ, N_WAY, MS], DT.float32, tag="ssb")
                smax8 = sm.tile([MQ, 8], DT.float32, tag="smax8")
                nc.vector.memset(smax8[:, N_WAY:], NEG)
                for w in range(N_WAY):
                    pw = pps.tile([MQ, MS], DT.float32, tag="sbank")
                    for cc in range(CC):
                        nc.tensor.matmul(
                            pw, qf[:, cc], sf[:, cc, w],
                            start=(cc == 0), stop=(cc == CC - 1),
                        )
                    nc.scalar.copy(s_sb[:, w], pw)
                    nc.vector.tensor_reduce(
                        smax8[:, w:w + 1], pw, axis=mybir.AxisListType.X, op=OP.max
                    )

                # top-2 over classes, scaled diff, global argmax (nearest)
                top8 = sm.tile([MQ, 8], DT.float32, tag="top8")
                nc.vector.max(out=top8, in_=smax8)
                nd2 = sm.tile([MQ, 2], DT.float32, tag="nd2")
                nc.vector.scalar_tensor_tensor(
                    out=nd2[:, 1:2], in0=top8[:, 0:1], scalar=top8[:, 1:2],
                    in1=rq2, op0=OP.subtract, op1=OP.mult,
                )
                idx8 = sm.tile([MQ, 8], DT.uint32, tag="idx8")
                nc.vector.max_index(idx8, top8, s_sb.rearrange("m w s -> m (w s)"))
                nc.vector.tensor_copy(nd2[:, 0:1], idx8[:, 0:1])

                # broadcast nearest/diff along partitions via PE
                nd2t = ppt.tile([1, 2 * MQ], DT.float32, tag="tiny")
                nc.tensor.transpose(nd2t[:, 0:MQ], nd2[:, 0:1], ident)
                nc.tensor.transpose(nd2t[:, MQ:], nd2[:, 1:2], ident)
                ndrow = sm.tile([1, 2 * MQ], DT.float32, tag="ndrow")
                nc.scalar.copy(ndrow, nd2t)
                ndbp = ppt.tile([MQ, 2 * MQ], DT.float32, tag="tiny")
                nc.tensor.matmul(ndbp, ones1, ndrow, start=True, stop=True)
                ndb = sm.tile([MQ, 2 * MQ], DT.float32, tag="ndb")
                nc.scalar.copy(ndb, ndbp)

                # score[m, m'] = (nearest[m']==nearest[m]) * diff[m']
                score = sm.tile([MQ, MQ], DT.float32, tag="score")
                nc.vector.scalar_tensor_tensor(
                    out=score, in0=ndb[:, 0:MQ], scalar=nd2[:, 0:1],
                    in1=ndb[:, MQ:], op0=OP.is_equal, op1=OP.mult,
                )
                stop8 = sm.tile([MQ, 8], DT.float32, tag="stop8")
                nc.vector.max(out=stop8, in_=score)
                sidx8 = sm.tile([MQ, 8], DT.uint32, tag="sidx8")
                nc.vector.max_index(sidx8, stop8, score)
                winf = sm.tile([MQ, 1], DT.float32, tag="winf")
                nc.vector.tensor_copy(winf, sidx8[:, 0:1])
                masks = sm.tile([MQ, 1], DT.float32, tag="masks")
                nc.vector.scalar_tensor_tensor(
                    out=masks, in0=winf, scalar=iota_f, in1=rq2,
                    op0=OP.is_equal, op1=OP.mult,
                )

                # predict[w] = sum_m masks[m] * smax[m, w]
                pred = ppt.tile([1, N_WAY], DT.float32, tag="tiny")
                nc.tensor.matmul(
                    pred, masks, smax8[:, 0:N_WAY], start=True, stop=True
                )
                nc.scalar.copy(prow[:, q], pred)

            # ---- epilogue: per-query -loss contributions ----
            pmax = outp.tile([1, NQ], DT.float32, tag="pmax")
            nc.vector.tensor_reduce(pmax, prow, axis=mybir.AxisListType.X, op=OP.max)
            tcen = outp.tile([1, NQ, N_WAY], DT.float32, tag="tcen")
            nc.vector.tensor_sub(tcen, prow, pmax.to_broadcast([1, NQ, N_WAY]))
            esum = outp.tile([1, NQ], DT.float32, tag="esum")
            ee = outp.tile([1, NQ, N_WAY], DT.float32, tag="ee")
            nc.scalar.activation(ee, tcen, AF.Exp)
            nc.vector.tensor_reduce(esum, ee, axis=mybir.AxisListType.X, op=OP.add)
            lse = outp.tile([1, NQ], DT.float32, tag="lse")
            nc.scalar.activation(lse, esum, AF.Ln)
            py = outp.tile([1, NQ], DT.float32, tag="py")
            tg = outp.tile([1, NQ, N_WAY], DT.float32, tag="tg")
            nc.vector.tensor_mul(
                tg, tcen, oneh_s.rearrange("o (q w) -> o q w", w=N_WAY)
            )
            nc.vector.tensor_reduce(py, tg, axis=mybir.AxisListType.X, op=OP.add)
            lossv = outp.tile([1, NQ], DT.float32, tag="lossv")
            nc.vector.tensor_sub(lossv, py, lse)
            nc.sync.dma_start(out=loss_d[:], in_=lossv)

    nc.compile()
    return nc


def shard_inputs(support_xf, query_xf, query_y):
    """Full inputs -> per-core input dicts (8 cores)."""
    support_xf = np.ascontiguousarray(support_xf, dtype=np.float32)
    query_xf = np.ascontiguousarray(query_xf, dtype=np.float32)
    query_y = np.asarray(query_y)
    in_maps = []
    for core in range(8):
        b = core // 2
        qs = 0 if core % 2 == 0 else Q - NQ  # 0 or 37
        sup = support_xf[b].reshape(N_WAY * K_SHOT, C, HW)
        qry = query_xf[b, qs:qs + NQ].reshape(NQ, C, HW)
        y = query_y[b, qs:qs + NQ].astype(np.int64)
        oneh = np.zeros((NQ, N_WAY), dtype=np.float32)
        oneh[np.arange(NQ), y] = 1.0
        in_maps.append({
            "sup": np.ascontiguousarray(sup),
            "qry": np.ascontiguousarray(qry),
            "oneh": oneh.reshape(1, NQ * N_WAY),
        })
    return in_maps


def gather_loss(results):
    """Per-core [1, NQ] -logp rows -> scalar mean loss."""
    total = 0.0
    for core in range(8):
        row = np.asarray(results[core]["loss"]).reshape(NQ)
        take = row if core % 2 == 0 else row[NQ - (Q - NQ):]  # drop overlap
        total += float(take.sum())
    return np.float32(-total / (B * Q))


_CACHED = {}


def kernel(support_xf, support_y, query_xf, query_y):
    key = "nc"
    if key not in _CACHED:
        _CACHED[key] = build_kernel()
    nc = _CACHED[key]
    in_maps = shard_inputs(support_xf, query_xf, query_y)
    res = run_bass_kernel_spmd(nc, in_maps, list(range(8)))
    return gather_loss(res.results)


if __name__ == "__main__":
    rng = np.random.default_rng(0)
    sup = rng.standard_normal((B, 25, C, 10, 10), dtype=np.float32)
    qry = rng.standard_normal((B, Q, C, 10, 10), dtype=np.float32)
    sy = rng.integers(0, N_WAY, (B, 25))
    qy = rng.integers(0, N_WAY, (B, Q))
    print(kernel(sup, sy, qry, qy))

